# revision 8
# baseline (speedup 1.0000x reference)
"""AdaptiveAttention Trainium2 kernel — 8 NeuronCores, SPMD.

Sharding: data-parallel over batch (2) x tensor-parallel over heads (4 groups
of 4 heads). Zero device collectives: the two cross-head reductions
(assembly normalization + raw halting update) and the dehead partial-output
sum are folded into the host-side unshard, which is exact (fp64) and removes
all ncfw latency floors.

Per-core math (b fixed, 4 local heads h):
  xT_hi/lo  = bf16 hi/lo split of query/key (transposed via DMA xbar through
              DRAM scratch), value hi only.
  cq|sq^T[e,q], ck|sk[k,e] : confidence/assembly projections in bf16x3
              (hi*Whi + hi*Wlo + lo*Whi) — fp32-grade accuracy, 1cyc/row.
  T_cs^T[e,q] = (Mbar @ [ck|sk])^T via 2 bf16 matmuls against exact 0/1 mask.
  conf/asm sums via selector matmul (fp32), sigmoid on ACT.
  aq^T/ak^T/v in plain bf16 (attention path tolerance ~1e-3).
  logits^T[k,q] = ak^T.T @ aq^T (K=64, head-pair row packing via base_partition)
  P = exp(logits) * Mbar^T  (no max-subtraction needed: |logits| < ~20)
  attn0|Z = [v|1].T @ P accumulate over k-tiles in PSUM
  attn0s = attn0 * conf*asm/(32*Z) (row-broadcast via rank-1 matmul)
  out_p[q,d] = attn0s.T @ w_dehead  (partial over local heads)
Outputs per core: out_p [2048,1024] f32, stats [2,2048] f32
  (stats = [sum_h asm_w, sum_h conf*asm_w]).
Host: raw_update = S1/(S0+1e-3); req/clamp/halting/residuals in fp64;
  final output = g * sum_cores(out_p) + output_acc + b_dehead.
"""
import numpy as np

B, S, D = 2, 2048, 1024
H, DH = 16, 64
NCORES = 8

_CACHE = {}


def _build_nc():
    import concourse.bacc as bacc
    import concourse.mybir as mybir
    from concourse.tile import TileContext

    f32 = mybir.dt.float32
    bf16 = mybir.dt.bfloat16
    u8 = mybir.dt.uint8
    AF = mybir.ActivationFunctionType
    ALU = mybir.AluOpType

    nc = bacc.Bacc()

    query = nc.declare_dram_parameter("query", [S, D], f32, isOutput=False)
    key = nc.declare_dram_parameter("key", [S, D], f32, isOutput=False)
    value = nc.declare_dram_parameter("value", [S, D], f32, isOutput=False)
    mask_u8 = nc.declare_dram_parameter("mask_u8", [S, S], u8, isOutput=False)
    w_qcs = nc.declare_dram_parameter("w_qcs", [D, 512], f32, isOutput=False)
    w_kcs = nc.declare_dram_parameter("w_kcs", [D, 512], f32, isOutput=False)
    w_aq = nc.declare_dram_parameter("w_aq", [D, 256], f32, isOutput=False)
    w_ak = nc.declare_dram_parameter("w_ak", [D, 256], f32, isOutput=False)
    w_v = nc.declare_dram_parameter("w_v", [D, 256], f32, isOutput=False)
    w_d = nc.declare_dram_parameter("w_d", [256, D], f32, isOutput=False)
    out_p = nc.declare_dram_parameter("out_p", [S, D], f32, isOutput=True)
    stats = nc.declare_dram_parameter("stats", [2, S], f32, isOutput=True)

    qhi_d = nc.dram_tensor("qhi_d", [S, D], bf16)
    qlo_d = nc.dram_tensor("qlo_d", [S, D], bf16)
    khi_d = nc.dram_tensor("khi_d", [S, D], bf16)
    klo_d = nc.dram_tensor("klo_d", [S, D], bf16)
    vhi_d = nc.dram_tensor("vhi_d", [S, D], bf16)
    mb_d = nc.dram_tensor("mb_d", [S, S], bf16)

    with TileContext(nc) as tc:
        with tc.tile_pool(name="wpool", bufs=1) as wpool, \
             tc.tile_pool(name="live", bufs=1) as live, \
             tc.tile_pool(name="stage", bufs=2) as stage:

            # persistent: attention projections + cq|sq (bf16 hi/lo)
            cqsq_hi = live.tile([128, 4 * S], bf16, tag="cqsqh")
            cqsq_lo = live.tile([128, 4 * S], bf16, tag="cqsql")
            aqT = live.tile([128, 2 * S], bf16, tag="aqT")
            akT = live.tile([128, 2 * S], bf16, tag="akT")
            sel_c = wpool.tile([128, 1], f32, tag="sel_c")
            sel_s = wpool.tile([128, 1], f32, tag="sel_s")
            nc.vector.memset(sel_c[:, :], 0.0)
            nc.vector.memset(sel_c[0:64, 0:1], 1.0)
            nc.vector.memset(sel_s[:, :], 0.0)
            nc.vector.memset(sel_s[64:128, 0:1], 1.0)
            ones1 = wpool.tile([1, 64], f32, tag="ones1")
            nc.vector.memset(ones1[:, :], 1.0)

            # ---------- P0: split q/k/v + mask to DRAM scratch ----------
            for src, hid, lod in ((query, qhi_d, qlo_d), (key, khi_d, klo_d),
                                  (value, vhi_d, None)):
                for qt in range(16):
                    xt = stage.tile([128, D], f32, tag="sf32")
                    nc.sync.dma_start(out=xt[:, :], in_=src[qt*128:(qt+1)*128, :])
                    hi = stage.tile([128, D], bf16, tag="shi")
                    nc.vector.tensor_copy(hi[:, :], xt[:, :])
                    nc.sync.dma_start(out=hid[qt*128:(qt+1)*128, :], in_=hi[:, :])
                    if lod is not None:
                        lo = stage.tile([128, D], bf16, tag="shi")
                        nc.vector.tensor_sub(lo[:, :], xt[:, :], hi[:, :])
                        nc.sync.dma_start(out=lod[qt*128:(qt+1)*128, :], in_=lo[:, :])
            for qt in range(16):
                mt8 = stage.tile([128, S], u8, tag="m8")
                nc.sync.dma_start(out=mt8[:, :], in_=mask_u8[qt*128:(qt+1)*128, :])
                mtb = stage.tile([128, S], bf16, tag="shi")
                nc.vector.tensor_copy(mtb[:, :], mt8[:, :])
                mtn = stage.tile([128, S], bf16, tag="shi")
                nc.vector.tensor_scalar(mtn[:, :], mtb[:, :], -1.0, 1.0,
                                        ALU.mult, ALU.add)
                nc.sync.dma_start(out=mb_d[qt*128:(qt+1)*128, :], in_=mtn[:, :])

            # ---------- Q group (q-halves) ----------
            with tc.tile_pool(name="qT", bufs=1) as pqT, \
                 tc.tile_pool(name="p2ps", bufs=4, space="PSUM") as p2ps:
                wqcs_hi = pqT.tile([128, 8 * 512], bf16, tag="wqcsh")
                wqcs_lo = pqT.tile([128, 8 * 512], bf16, tag="wqcsl")
                waq_sb = pqT.tile([128, 8 * 256], bf16, tag="waq")
                for dt in range(8):
                    st = stage.tile([128, D], f32, tag="sf32")
                    nc.sync.dma_start(out=st[:, 0:512], in_=w_qcs[dt*128:(dt+1)*128, :])
                    nc.vector.tensor_copy(wqcs_hi[:, dt*512:(dt+1)*512], st[:, 0:512])
                    nc.vector.tensor_sub(wqcs_lo[:, dt*512:(dt+1)*512], st[:, 0:512],
                                         wqcs_hi[:, dt*512:(dt+1)*512])
                    st2 = stage.tile([128, D], f32, tag="sf32")
                    nc.sync.dma_start(out=st2[:, 0:256], in_=w_aq[dt*128:(dt+1)*128, :])
                    nc.scalar.activation(waq_sb[:, dt*256:(dt+1)*256], st2[:, 0:256], AF.Copy)
                qThi = pqT.tile([128, 8 * 1024], bf16, tag="qThi")
                qTlo = pqT.tile([128, 8 * 1024], bf16, tag="qTlo")
                for qh in range(2):
                    for dt in range(8):
                        nc.sync.dma_start(out=qThi[:, dt*1024:(dt+1)*1024],
                                          in_=qhi_d[qh*1024:(qh+1)*1024, dt*128:(dt+1)*128],
                                          transpose=True)
                        nc.sync.dma_start(out=qTlo[:, dt*1024:(dt+1)*1024],
                                          in_=qlo_d[qh*1024:(qh+1)*1024, dt*128:(dt+1)*128],
                                          transpose=True)
                    for et in range(4):
                        pss = [p2ps.tile([128, 512], f32, tag="pj", name=f"pj{_i}")
                               for _i in range(2)]
                        for dt in range(8):
                            wh = wqcs_hi[:, dt*512 + et*128: dt*512 + (et+1)*128]
                            wl = wqcs_lo[:, dt*512 + et*128: dt*512 + (et+1)*128]
                            for q2 in range(2):
                                qhs = qThi[:, dt*1024 + q2*512: dt*1024 + (q2+1)*512]
                                qls = qTlo[:, dt*1024 + q2*512: dt*1024 + (q2+1)*512]
                                nc.tensor.matmul(pss[q2][:, :], wh, qhs, start=(dt == 0), stop=False)
                                nc.tensor.matmul(pss[q2][:, :], wl, qhs, start=False, stop=False)
                                nc.tensor.matmul(pss[q2][:, :], wh, qls, start=False, stop=(dt == 7))
                        for q2 in range(2):
                            qc = qh*2 + q2
                            sl = slice(et*S + qc*512, et*S + (qc+1)*512)
                            nc.scalar.activation(cqsq_hi[:, sl], pss[q2][:, :], AF.Copy)
                            nc.vector.tensor_sub(cqsq_lo[:, sl], pss[q2][:, :],
                                                 cqsq_hi[:, sl])
                    for et in range(2):
                        pss = [p2ps.tile([128, 512], f32, tag="pj", name=f"pja{_i}")
                               for _i in range(2)]
                        for dt in range(8):
                            wh = waq_sb[:, dt*256 + et*128: dt*256 + (et+1)*128]
                            for q2 in range(2):
                                qhs = qThi[:, dt*1024 + q2*512: dt*1024 + (q2+1)*512]
                                nc.tensor.matmul(pss[q2][:, :], wh, qhs,
                                                 start=(dt == 0), stop=(dt == 7))
                        for q2 in range(2):
                            qc = qh*2 + q2
                            nc.scalar.activation(aqT[:, et*S + qc*512: et*S + (qc+1)*512],
                                                 pss[q2][:, :], AF.Copy)

            with tc.tile_pool(name="ckskp", bufs=1) as ckskp:
                cksk_hi = ckskp.tile([128, 16 * 512], bf16, tag="ckskh")
                cksk_lo = ckskp.tile([128, 16 * 512], bf16, tag="ckskl")

                # ---------- K group (k-halves) ----------
                with tc.tile_pool(name="kT", bufs=1) as pkT, \
                     tc.tile_pool(name="p2psk", bufs=4, space="PSUM") as p2psk:
                    wkcs_hi = pkT.tile([128, 8 * 512], bf16, tag="wkcsh")
                    wkcs_lo = pkT.tile([128, 8 * 512], bf16, tag="wkcsl")
                    wak_sb = pkT.tile([128, 8 * 256], bf16, tag="wak")
                    for dt in range(8):
                        st = stage.tile([128, D], f32, tag="sf32")
                        nc.sync.dma_start(out=st[:, 0:512], in_=w_kcs[dt*128:(dt+1)*128, :])
                        nc.vector.tensor_copy(wkcs_hi[:, dt*512:(dt+1)*512], st[:, 0:512])
                        nc.vector.tensor_sub(wkcs_lo[:, dt*512:(dt+1)*512], st[:, 0:512],
                                             wkcs_hi[:, dt*512:(dt+1)*512])
                        st2 = stage.tile([128, D], f32, tag="sf32")
                        nc.sync.dma_start(out=st2[:, 0:256], in_=w_ak[dt*128:(dt+1)*128, :])
                        nc.scalar.activation(wak_sb[:, dt*256:(dt+1)*256], st2[:, 0:256], AF.Copy)
                    kThi = pkT.tile([128, 8 * 1024], bf16, tag="kThi")
                    kTlo = pkT.tile([128, 8 * 1024], bf16, tag="kTlo")
                    for kh in range(2):
                        for dt in range(8):
                            nc.sync.dma_start(out=kThi[:, dt*1024:(dt+1)*1024],
                                              in_=khi_d[kh*1024:(kh+1)*1024, dt*128:(dt+1)*128],
                                              transpose=True)
                            nc.sync.dma_start(out=kTlo[:, dt*1024:(dt+1)*1024],
                                              in_=klo_d[kh*1024:(kh+1)*1024, dt*128:(dt+1)*128],
                                              transpose=True)
                        for kt2 in range(8):
                            kt = kh*8 + kt2
                            ps = p2psk.tile([128, 512], f32, tag="pj2")
                            for dt in range(8):
                                kh_s = kThi[:, dt*1024 + kt2*128: dt*1024 + (kt2+1)*128]
                                kl_s = kTlo[:, dt*1024 + kt2*128: dt*1024 + (kt2+1)*128]
                                wh = wkcs_hi[:, dt*512:(dt+1)*512]
                                wl = wkcs_lo[:, dt*512:(dt+1)*512]
                                nc.tensor.matmul(ps[:, :], kh_s, wh, start=(dt == 0), stop=False)
                                nc.tensor.matmul(ps[:, :], kh_s, wl, start=False, stop=False)
                                nc.tensor.matmul(ps[:, :], kl_s, wh, start=False, stop=(dt == 7))
                            nc.scalar.activation(cksk_hi[:, kt*512:(kt+1)*512], ps[:, :], AF.Copy)
                            nc.vector.tensor_sub(cksk_lo[:, kt*512:(kt+1)*512], ps[:, :],
                                                 cksk_hi[:, kt*512:(kt+1)*512])
                        for et in range(2):
                            pss = [p2psk.tile([128, 512], f32, tag="pj2", name=f"pjk{_i}")
                                   for _i in range(2)]
                            for dt in range(8):
                                wh = wak_sb[:, dt*256 + et*128: dt*256 + (et+1)*128]
                                for k2 in range(2):
                                    khs = kThi[:, dt*1024 + k2*512: dt*1024 + (k2+1)*512]
                                    nc.tensor.matmul(pss[k2][:, :], wh, khs,
                                                     start=(dt == 0), stop=(dt == 7))
                            for k2 in range(2):
                                kc = kh*2 + k2
                                nc.scalar.activation(akT[:, et*S + kc*512: et*S + (kc+1)*512],
                                                     pss[k2][:, :], AF.Copy)

                with tc.tile_pool(name="vpool", bufs=1) as vpool:
                    v_sb = vpool.tile([128, 16 * 260], bf16, tag="v_sb")
                    # ---------- V group (k-halves) ----------
                    with tc.tile_pool(name="vT", bufs=1) as pvT, \
                         tc.tile_pool(name="p2psv", bufs=2, space="PSUM") as p2psv:
                        wv_sb = pvT.tile([128, 8 * 256], bf16, tag="wv")
                        for dt in range(8):
                            st = stage.tile([128, D], f32, tag="sf32")
                            nc.sync.dma_start(out=st[:, 0:256], in_=w_v[dt*128:(dt+1)*128, :])
                            nc.scalar.activation(wv_sb[:, dt*256:(dt+1)*256], st[:, 0:256], AF.Copy)
                        vThi = pvT.tile([128, 8 * 1024], bf16, tag="vThi")
                        for kh in range(2):
                            for dt in range(8):
                                nc.sync.dma_start(out=vThi[:, dt*1024:(dt+1)*1024],
                                                  in_=vhi_d[kh*1024:(kh+1)*1024, dt*128:(dt+1)*128],
                                                  transpose=True)
                            for kt2 in range(8):
                                kt = kh*8 + kt2
                                ps = p2psv.tile([128, 256], f32, tag="pjv")
                                for dt in range(8):
                                    vh = vThi[:, dt*1024 + kt2*128: dt*1024 + (kt2+1)*128]
                                    wh = wv_sb[:, dt*256:(dt+1)*256]
                                    nc.tensor.matmul(ps[:, :], vh, wh, start=(dt == 0), stop=(dt == 7))
                                blk = v_sb[:, kt*260:(kt+1)*260].rearrange("p (h x) -> p h x", x=65)
                                nc.vector.tensor_copy(
                                    blk[:, :, 0:64],
                                    ps[:, :].rearrange("p (h e) -> p h e", e=64))
                                nc.vector.memset(blk[:, :, 64:65], 1.0)

                    with tc.tile_pool(name="small", bufs=1) as small:
                        fa = [small.tile([1, S], bf16, tag=f"fa{h}", name=f"fa{h}")
                              for h in range(4)]
                        stats0 = small.tile([1, S], f32, tag="stats0")
                        stats1 = small.tile([1, S], f32, tag="stats1")

                        # ---------- P3: T_cs + sigmoids + stats ----------
                        with tc.tile_pool(name="p3ps", bufs=1, space="PSUM") as p3ps, \
                             tc.tile_pool(name="p3st", bufs=2, space="PSUM") as p3st, \
                             tc.tile_pool(name="p3sb", bufs=2) as p3sb:
                            for qc in range(4):
                                tcs = [p3ps.tile([128, 512], f32, tag=f"tcs{h}", name=f"tcs{h}")
                                       for h in range(4)]
                                for ks in range(16):
                                    mt = p3sb.tile([128, 512], bf16, tag="mt")
                                    nc.sync.dma_start(out=mt[:, :],
                                                      in_=mb_d[qc*512:(qc+1)*512, ks*128:(ks+1)*128],
                                                      transpose=True)
                                    for h in range(4):
                                        sh = cksk_hi[:, ks*512 + h*128: ks*512 + (h+1)*128]
                                        sl = cksk_lo[:, ks*512 + h*128: ks*512 + (h+1)*128]
                                        nc.tensor.matmul(tcs[h][:, :], sh, mt[:, :],
                                                         start=(ks == 0), stop=False)
                                        nc.tensor.matmul(tcs[h][:, :], sl, mt[:, :],
                                                         start=False, stop=(ks == 15))
                                for h in range(4):
                                    qsl = slice(h*S + qc*512, h*S + (qc+1)*512)
                                    U = p3sb.tile([128, 512], f32, tag="U")
                                    nc.vector.tensor_mul(U[:, :], tcs[h][:, :], cqsq_hi[:, qsl])
                                    U2 = p3sb.tile([128, 512], f32, tag="U2")
                                    nc.vector.tensor_mul(U2[:, :], tcs[h][:, :], cqsq_lo[:, qsl])
                                    nc.vector.tensor_add(U[:, :], U[:, :], U2[:, :])
                                    stpc = p3st.tile([1, 512], f32, tag="stpc")
                                    nc.tensor.matmul(stpc[:, :], sel_c[:, :], U[:, :],
                                                     start=True, stop=True)
                                    stps = p3st.tile([1, 512], f32, tag="stps")
                                    nc.tensor.matmul(stps[:, :], sel_s[:, :], U[:, :],
                                                     start=True, stop=True)
                                    cfc = p3sb.tile([1, 512], f32, tag="cfc")
                                    nc.scalar.activation(cfc[:, :], stpc[:, :], AF.Sigmoid)
                                    asc = p3sb.tile([1, 512], f32, tag="asc")
                                    nc.scalar.activation(asc[:, :], stps[:, :], AF.Sigmoid)
                                    cac = p3sb.tile([1, 512], f32, tag="cac")
                                    nc.vector.tensor_mul(cac[:, :], cfc[:, :], asc[:, :])
                                    qs = slice(qc*512, (qc+1)*512)
                                    nc.vector.tensor_scalar_mul(fa[h][:, qs], cac[:, :], 1.0/32.0)
                                    if h == 0:
                                        nc.vector.tensor_copy(stats0[:, qs], asc[:, :])
                                        nc.vector.tensor_copy(stats1[:, qs], cac[:, :])
                                    else:
                                        nc.vector.tensor_add(stats0[:, qs], stats0[:, qs], asc[:, :])
                                        nc.vector.tensor_add(stats1[:, qs], stats1[:, qs], cac[:, :])
                            nc.sync.dma_start(out=stats[0:1, :], in_=stats0[:, :])
                            nc.sync.dma_start(out=stats[1:2, :], in_=stats1[:, :])

                        # ---------- P4: attention + dehead ----------
                        with tc.tile_pool(name="p4ps", bufs=1, space="PSUM") as p4ps, \
                             tc.tile_pool(name="p4lg", bufs=2, space="PSUM") as p4lg, \
                             tc.tile_pool(name="p4od", bufs=2, space="PSUM") as p4od, \
                             tc.tile_pool(name="p4sb", bufs=3) as p4sb, \
                             tc.tile_pool(name="p4w", bufs=1) as p4w, \
                             tc.tile_pool(name="p4out", bufs=2) as p4out:
                            wd_sb = p4w.tile([128, 2 * D], bf16, tag="wd")
                            for et in range(2):
                                st = stage.tile([128, D], f32, tag="sf32")
                                nc.sync.dma_start(out=st[:, :], in_=w_d[et*128:(et+1)*128, :])
                                nc.scalar.activation(wd_sb[:, et*D:(et+1)*D], st[:, :], AF.Copy)
                            for qc in range(4):
                                avs = [p4ps.tile([65, 512], f32, tag=f"av{h}", name=f"av{h}")
                                       for h in range(4)]
                                for ks in range(16):
                                    mt = p4sb.tile([128, 512], bf16, tag="mt4")
                                    nc.sync.dma_start(out=mt[:, :],
                                                      in_=mb_d[qc*512:(qc+1)*512, ks*128:(ks+1)*128],
                                                      transpose=True)
                                    for h in range(4):
                                        et, hb = h // 2, (h % 2) * 64
                                        lg = p4lg.tile([128, 512], f32, tag="lg")
                                        nc.tensor.matmul(
                                            lg[:, :],
                                            akT[hb:hb+64, et*S + ks*128: et*S + (ks+1)*128],
                                            aqT[hb:hb+64, et*S + qc*512: et*S + (qc+1)*512],
                                            start=True, stop=True)
                                        pt = p4sb.tile([128, 512], bf16, tag="pt")
                                        nc.scalar.activation(pt[:, :], lg[:, :], AF.Exp)
                                        ptm = p4sb.tile([128, 512], bf16, tag="ptm")
                                        nc.vector.tensor_mul(ptm[:, :], pt[:, :], mt[:, :])
                                        nc.tensor.matmul(avs[h][:, :],
                                                         v_sb[:, (ks*4+h)*65:(ks*4+h+1)*65],
                                                         ptm[:, :],
                                                         start=(ks == 0), stop=(ks == 15))
                                attn0s = p4sb.tile([128, 2 * 512], bf16, tag="a0s")
                                for h in range(4):
                                    et, hb = h // 2, (h % 2) * 64
                                    rz = p4sb.tile([1, 512], f32, tag="rz")
                                    nc.vector.reciprocal(rz[:, :], avs[h][64:65, :])
                                    sc = p4sb.tile([1, 512], f32, tag="sc")
                                    nc.vector.tensor_mul(sc[:, :], rz[:, :],
                                                         fa[h][:, qc*512:(qc+1)*512])
                                    bc = p4lg.tile([64, 512], f32, tag="lg")
                                    nc.tensor.matmul(bc[:, :], ones1[:, :], sc[:, :],
                                                     start=True, stop=True)
                                    bcs = p4sb.tile([64, 512], f32, tag="bcs")
                                    nc.scalar.activation(bcs[:, :], bc[:, :], AF.Copy)
                                    nc.vector.tensor_mul(attn0s[hb:hb+64, et*512:(et+1)*512],
                                                         avs[h][0:64, :], bcs[:, :])
                                for qs in range(4):
                                    for dc in range(2):
                                        od = p4od.tile([128, 512], f32, tag="od")
                                        for et in range(2):
                                            nc.tensor.matmul(
                                                od[:, :],
                                                attn0s[:, et*512 + qs*128: et*512 + (qs+1)*128],
                                                wd_sb[:, et*D + dc*512: et*D + (dc+1)*512],
                                                start=(et == 0), stop=(et == 1))
                                        osb = p4out.tile([128, 512], f32, tag="ot")
                                        nc.scalar.activation(osb[:, :], od[:, :], AF.Copy)
                                        nc.sync.dma_start(
                                            out=out_p[qc*512 + qs*128: qc*512 + (qs+1)*128,
                                                      dc*512:(dc+1)*512],
                                            in_=osb[:, :])
    nc.finalize()
    return nc


def _get_runner():
    """Build + compile once; return a callable in_maps -> list of result dicts."""
    if "runner" in _CACHE:
        return _CACHE["runner"]
    import jax
    import jax.numpy as jnp
    from jax.sharding import Mesh, PartitionSpec
    from jax.experimental.shard_map import shard_map
    import concourse.mybir as mybir
    from concourse import bass2jax
    from concourse.bass2jax import _bass_exec_p, install_neuronx_cc_hook, \
        partition_id_tensor

    nc = _build_nc()
    install_neuronx_cc_hook()

    partition_name = nc.partition_id_tensor.name if nc.partition_id_tensor else None
    in_names, out_names, out_avals, zero_outs = [], [], [], []
    for alloc in nc.m.functions[0].allocations:
        if not isinstance(alloc, mybir.MemoryLocationSet):
            continue
        name = alloc.memorylocations[0].name
        if alloc.kind == "ExternalInput":
            if name != partition_name:
                in_names.append(name)
        elif alloc.kind == "ExternalOutput":
            shape = tuple(alloc.tensor_shape)
            dtype = mybir.dt.np(alloc.dtype)
            out_names.append(name)
            out_avals.append(jax.core.ShapedArray(shape, dtype))
            zero_outs.append(np.zeros(shape, dtype))
    n_params = len(in_names)
    n_outs = len(out_avals)
    all_names = in_names + out_names
    if partition_name is not None:
        all_names = all_names + [partition_name]
    donate = tuple(range(n_params, n_params + n_outs))

    def _body(*args):
        operands = list(args)
        if partition_name is not None:
            operands.append(partition_id_tensor())
        outs = _bass_exec_p.bind(
            *operands, out_avals=tuple(out_avals), in_names=tuple(all_names),
            out_names=tuple(out_names), lowering_input_output_aliases=(),
            sim_require_finite=True, sim_require_nnan=True, nc=nc)
        return tuple(outs)

    devices = jax.devices()[:NCORES]
    mesh = Mesh(np.asarray(devices), ("core",))
    in_specs = (PartitionSpec("core"),) * (n_params + n_outs)
    out_specs = (PartitionSpec("core"),) * n_outs
    sharded = jax.jit(
        shard_map(_body, mesh=mesh, in_specs=in_specs, out_specs=out_specs,
                  check_rep=False),
        donate_argnums=donate, keep_unused=True)

    def run(in_maps):
        concat_in = [np.concatenate([np.asarray(m[k]) for m in in_maps], axis=0)
                     for k in in_names]
        concat_zeros = [np.zeros((NCORES * z.shape[0], *z.shape[1:]), z.dtype)
                        for z in zero_outs]
        out_arrs = sharded(*concat_in, *concat_zeros)
        out_np = [np.asarray(a) for a in out_arrs]
        return [
            {name: out_np[i].reshape(NCORES, *out_avals[i].shape)[c]
             for i, name in enumerate(out_names)}
            for c in range(NCORES)
        ]

    _CACHE["runner"] = run
    return run


def _prep_in_maps(inputs):
    q = np.ascontiguousarray(inputs["query"], dtype=np.float32)
    k = np.ascontiguousarray(inputs["key"], dtype=np.float32)
    v = np.ascontiguousarray(inputs["value"], dtype=np.float32)
    mask = np.asarray(inputs["mask"])
    wq_c, wq_s = np.asarray(inputs["wq_c"], np.float32), np.asarray(inputs["wq_s"], np.float32)
    wk_c, wk_s = np.asarray(inputs["wk_c"], np.float32), np.asarray(inputs["wk_s"], np.float32)
    wq_a, wk_a = np.asarray(inputs["wq_a"], np.float32), np.asarray(inputs["wk_a"], np.float32)
    wv = np.asarray(inputs["wv"], np.float32)
    wd = np.asarray(inputs["w_dehead"], np.float32)

    in_maps = []
    for c in range(NCORES):
        b, hg = c // 4, c % 4
        hs = hg * 4
        in_maps.append({
            "query": q[b], "key": k[b], "value": v[b],
            "mask_u8": np.ascontiguousarray(mask[b, 0].astype(np.uint8)),
            "w_qcs": np.ascontiguousarray(
                np.concatenate([wq_c[:, hs:hs+4], wq_s[:, hs:hs+4]], axis=2
                               ).reshape(D, 512)),
            "w_kcs": np.ascontiguousarray(
                np.concatenate([wk_c[:, hs:hs+4], wk_s[:, hs:hs+4]], axis=2
                               ).reshape(D, 512)),
            "w_aq": np.ascontiguousarray(wq_a[:, hs:hs+4].reshape(D, 256)),
            "w_ak": np.ascontiguousarray(wk_a[:, hs:hs+4].reshape(D, 256)),
            "w_v": np.ascontiguousarray(wv[:, hs:hs+4].reshape(D, 256)),
            "w_d": np.ascontiguousarray(wd[hs:hs+4].reshape(256, D)),
        })
    return in_maps


def _post(results, inputs):
    halting = np.asarray(inputs["halting"], np.float64)
    residuals = np.asarray(inputs["residuals"], np.float32)
    output_acc = np.asarray(inputs["output_acc"], np.float32)
    b_dehead = np.asarray(inputs["b_dehead"], np.float32)

    halting_out = np.empty((B, S), np.float32)
    residuals_out = np.empty((B, S), np.float32)
    output_out = np.empty((B, S, D), np.float32)
    for b in range(2):
        cores = [results[b * 4 + hg] for hg in range(4)]
        S0 = np.sum([c["stats"][0].astype(np.float64) for c in cores], axis=0)
        S1 = np.sum([c["stats"][1].astype(np.float64) for c in cores], axis=0)
        raw_update = S1 / (S0 + 0.001)
        raw_new = raw_update + halting[b]
        req = (raw_new > 1.0 - 0.001) & (raw_new != 1.0)
        clamp = (1.0 - halting[b]) / (raw_update + 1e-12)
        g = np.where(req, clamp, 1.0)
        halt_update = np.where(req, clamp * raw_update, raw_update)
        halting_out[b] = (halting[b] + halt_update).astype(np.float32)
        residuals_out[b] = residuals[b] + np.where(req, clamp * raw_update, 0.0
                                                   ).astype(np.float32)
        psum = np.sum([c["out_p"].astype(np.float64) for c in cores], axis=0)
        output_out[b] = (g[:, None] * psum).astype(np.float32) \
            + output_acc[b] + b_dehead[None, :]
    return halting_out, residuals_out, output_out


def kernel(**inputs):
    run = _get_runner()
    in_maps = _prep_in_maps(inputs)
    results = run(in_maps)
    return _post(results, inputs)
